# revision 54
# baseline (speedup 1.0000x reference)
"""Trainium2 Bass kernel for conditional-adjustment conv (CAConv).

Per sample b: h = relu(c[b] @ mlp_w1 + mlp_b1); adj = h @ mlp_w2 + mlp_b2;
w[b] = conv_w + adj.reshape(Co,Ci,3,3); out[b] = conv2d(x[b], w[b], pad=1) + conv_b.

Sharding: data-parallel over batch, 4 samples per core on 8 cores (SPMD).

Per-core schedule (bf16 operands, fp32 PSUM accumulate):
  t=0: gpsimd memsets (wblk/hts/warm); sync loads packed consts + the two
  tap-grouped w2 tables; scalar (ACT HWDGE) streams padded-x chunks.
  The tensor engine runs ~9 throwaway warmup matmuls on a zeroed tile so
  the PE_HAM clock gate promotes 4/8 -> 8/8 during the DMA wait instead of
  during the conv.
  Stage A (weight gen): ph = w1'.T @ ct' (ones-row trick), relu -> ht[17,4].
  ht is scattered into block-diagonal stationaries htsA[119,28]/htsB[34,8]
  so ONE matmul chunk computes all 4 samples x 7 (resp 2) taps at once:
  adjA[7b+t', ci*64+co] = per-sample conv weight (w2 row 16 carries
  mlp_b2 + conv_w so the matmul output IS the complete weight). 8192
  streamed columns total vs 36864 for the naive [17,4] stationary.
  PSUM chunks are copied (vector/scalar/gpsimd round-robin) to bf16 SBUF,
  then 36 small DMAs scatter each (sample, tap) [ci,co] block onto the
  diagonal of the per-pair block-diag weight wblk[ci+64h, t*128+64h+co],
  spread tap-major across the 3 DMA-trigger queues so conv tap t unblocks
  early.
  Stage B (conv): identical structure to the proven baseline - per pair,
  po[128,512] chunks (2 samples x 64 co, 4 h-rows x 128 w) accumulate 9
  shift-tap K=128 matmuls. Bias-add on vector into [128,2048] group tiles,
  one 1MB store DMA per group of 4 chunks (sync queue).
"""

import sys

if "/opt/trn_rl_repo" not in sys.path:
    sys.path.insert(0, "/opt/trn_rl_repo")

import numpy as np
import ml_dtypes

BF16 = ml_dtypes.bfloat16

B = 32
NCORES = 8
BPC = B // NCORES          # samples per core = 4
PAIRS = BPC // 2           # sample pairs per core = 2
CIN = COUT = 64
H = W = 128
HP = WP = 130              # padded dims
KH = KW = 3
NT = KH * KW               # taps = 9
CL = 8                     # c length
CL1 = CL + 1               # + ones row
MH = 16                    # mlp hidden
K2 = MH + 1                # mlp hidden + ones row
NTA = 7                    # taps in stage-A group A
NTB = NT - NTA             # taps in group B = 2
KA = NTA * K2              # 119 partitions
KB = NTB * K2              # 34 partitions
MA = NTA * BPC             # 28 psum partitions
MB = NTB * BPC             # 8
CC = CIN * COUT            # 4096 cols per tap
NCH = (H * W) // 512       # 512-col output chunks per pair = 32
NGRP = NCH // 4            # store groups of 4 chunks = 8
NXC = 5                    # x chunks per pair (26 padded rows each)
XCE = (HP * WP) // NXC     # 3380 elements per chunk row

_CACHE = {}


def _build():
    import concourse.bass as bass
    import concourse.mybir as mybir
    import concourse.tile as tile
    from concourse import bacc
    from concourse.tile_rust import add_dep_helper

    f32 = mybir.dt.float32
    bf16 = mybir.dt.bfloat16
    AF = mybir.ActivationFunctionType

    nc = bacc.Bacc("TRN2", target_bir_lowering=False, debug=False)

    xs_d = nc.dram_tensor("xsp", [BPC, CIN, HP * WP], bf16, kind="ExternalInput")
    cst_d = nc.dram_tensor("cst", [128, 162], f32, kind="ExternalInput")
    w2a_d = nc.dram_tensor("w2a", [KA, CC], bf16, kind="ExternalInput")
    w2b_d = nc.dram_tensor("w2b", [KB, CC], bf16, kind="ExternalInput")
    out_d = nc.dram_tensor("out", [BPC, COUT, H, W], f32, kind="ExternalOutput")

    with tile.TileContext(nc) as tc:
        with (
            tc.tile_pool(name="consts", bufs=1) as consts,
            tc.tile_pool(name="xpool", bufs=1) as xpool,
            tc.tile_pool(name="opool", bufs=3) as opool,
            tc.tile_pool(name="pspool", bufs=1, space=bass.MemorySpace.PSUM) as ps,
        ):
            # ---- tiles ----
            cst_sb = consts.tile([128, 162], f32)
            warm = consts.tile([128, 512], bf16)
            htv = consts.tile([KA, BPC], bf16)
            htsA = consts.tile([KA, MA], bf16)
            htsB = consts.tile([KB, MB], bf16)
            w2a_sb = consts.tile([KA, CC], bf16)
            w2b_sb = consts.tile([KB, CC], bf16)
            adjA = consts.tile([MA, CC], bf16)
            adjB = consts.tile([MB, CC], bf16)
            wblk = [
                consts.tile([128, NT * 128], bf16, name=f"wblk{p}") for p in range(PAIRS)
            ]
            xps = [
                xpool.tile([128, HP * WP], bf16, name=f"xp{p}") for p in range(PAIRS)
            ]

            # ---- t=0: gpsimd memsets (warm first: it gates the PE warmup) ----
            nc.gpsimd.memset(warm[:], 0.0)
            nc.gpsimd.memset(wblk[0][:], 0.0)
            nc.gpsimd.memset(wblk[1][:], 0.0)

            # ---- t=0: const + w2 table loads. One dma_start only spreads
            # over ~7 DMA engines (descriptor round-robin), so w2a is split
            # across BOTH HWDGE queues (sync + scalar) and w2b goes to the
            # software DGE — maximizing engine parallelism for the transfer
            # that gates stage A. Tile reorders queues freely, so pacing
            # below uses explicit dep edges, not emission order. ----
            # 10-row chunks: each ~80KB => one descriptor => one DMA engine,
            # so 12 chunks across both HWDGE queues engage ~12 engines in
            # parallel instead of the ~7 a monolithic transfer gets
            nc.sync.dma_start(out=cst_sb[:], in_=cst_d.ap())
            iw2a = None
            for r0 in range(0, 60, 10):
                iw2a = nc.sync.dma_start(
                    out=w2a_sb[r0 : r0 + 10, :], in_=w2a_d.ap()[r0 : r0 + 10, :]
                )
            for r0 in range(60, KA, 10):
                r1 = min(r0 + 10, KA)
                iw2a = nc.scalar.dma_start(
                    out=w2a_sb[r0:r1, :], in_=w2a_d.ap()[r0:r1, :]
                )
            nc.sync.dma_start(out=w2b_sb[0:17, :], in_=w2b_d.ap()[0:17, :])
            iw2b = nc.scalar.dma_start(out=w2b_sb[17:KB, :], in_=w2b_d.ap()[17:KB, :])

            def pace(after, inst, why):
                # add_dep_helper(from, to) == "from depends on to":
                # make `inst` wait until `after` has issued
                add_dep_helper(inst.ins, after.ins, sync=True, reason=why)
                return inst

            # ---- x loads on scalar (ACT HWDGE), paced behind `after` so
            # bulk x packets don't flood the rings ahead of small latency-
            # critical transfers. Chunks are element ranges of the padded
            # [130x130] plane: fine-grained first chunks for pair 0 so the
            # conv can start early, coarse elsewhere for packet efficiency.
            XC0 = [0, 1690, 3380, 6760, 10140, 13520, 16900]      # pair 0
            XC1 = [0, 3380, 6760, 10140, 13520, 16900]            # pair 1
            def load_x_chunk(p, e0, e1, after):
                inst = nc.scalar.dma_start(
                    out=xps[p][:, e0:e1],
                    in_=xs_d.ap()[2 * p : 2 * p + 2][:, :, e0:e1],
                )
                return pace(after, inst, "pace bulk x")

            # ---- PE warmup: promote the HAM clock gate while DMAs fly ----
            def warmup(i):
                psw = ps.tile([128, 512], f32, tag="ps", bufs=8, name=f"psw{i}")
                nc.tensor.matmul(
                    psw[:], warm[:, 0:128], warm[:], start=True, stop=True
                )

            for i in range(6):
                warmup(i)
            NWARM_PRE = 22

            # ---- stage A: conditioning MLP. The ph matmul uses w1 columns
            # replicated 7x (host-packed) so its output [119, 4] already sits
            # at every tap-group's partition range; one relu then one masked
            # broadcast-multiply per group builds htsA/htsB with NO DMAs —
            # immune to DMA-ring congestion (mask rides in the cst load). ----
            ph = ps.tile([KA, BPC], f32, tag="ps", bufs=8)
            nc.tensor.matmul(
                ph[:], cst_sb[0:CL1, 4 : 4 + KA], cst_sb[0:CL1, 0:4],
                start=True, stop=True,
            )
            for i in range(6, NWARM_PRE):
                warmup(i)

            nc.scalar.activation(
                out=htv[:], in_=ph[:], func=AF.Relu, bias=cst_sb[0:KA, 123:124]
            )
            nc.vector.tensor_tensor(
                htsA[:].rearrange("p (t b) -> p t b", t=NTA),
                htv[:].unsqueeze(1).broadcast_to([KA, NTA, BPC]),
                cst_sb[0:KA, 126 : 126 + MA].rearrange("p (t b) -> p t b", t=NTA),
                mybir.AluOpType.mult,
            )
            nc.vector.tensor_tensor(
                htsB[:].rearrange("p (t b) -> p t b", t=NTB),
                htv[0:KB, :].unsqueeze(1).broadcast_to([KB, NTB, BPC]),
                cst_sb[0:KB, 154 : 154 + MB].rearrange("p (t b) -> p t b", t=NTB),
                mybir.AluOpType.mult,
            )

            load_x_chunk(0, XC0[0], XC0[1], iw2a)
            load_x_chunk(0, XC0[1], XC0[2], iw2a)
            for k in range(2, len(XC0) - 1):
                load_x_chunk(0, XC0[k], XC0[k + 1], iw2b)

            # stage-A matmuls: 16 chunks of 512 cols; copies alternate
            # vector/scalar so neither engine gates the tail (gpsimd
            # cannot read PSUM)
            def copy_chunk(i, dst, src):
                if i % 2 == 0:
                    nc.vector.tensor_copy(dst, src)
                else:
                    nc.scalar.activation(out=dst, in_=src, func=AF.Copy)

            for c in range(CC // 512):
                pa = ps.tile([MA, 512], f32, tag="ps", bufs=8, name=f"paA{c}")
                nc.tensor.matmul(
                    pa[:],
                    htsA[:, :],
                    w2a_sb[:, c * 512 : (c + 1) * 512],
                    start=True,
                    stop=True,
                )
                copy_chunk(c, adjA[:, c * 512 : (c + 1) * 512], pa[:])
            for c in range(CC // 512):
                pb = ps.tile([MB, 512], f32, tag="ps", bufs=8, name=f"paB{c}")
                nc.tensor.matmul(
                    pb[:],
                    htsB[:, :],
                    w2b_sb[:, c * 512 : (c + 1) * 512],
                    start=True,
                    stop=True,
                )
                copy_chunk(c + 1, adjB[:, c * 512 : (c + 1) * 512], pb[:])

            # scatter per-sample weights onto wblk's diagonal. One DMA per
            # (sample, tap-group) covers ALL its taps: dst is the wblk AP
            # rearranged to [taps, ci, co] (partition dim transposed inward)
            # matching the adj source row-block iteration — 8 triggers
            # instead of 36. Pair-0's samples go first so the conv can start
            # while pair-1's weights are still scattering.
            qeng = [nc.sync, nc.scalar, nc.gpsimd]
            scat_last = None
            i = 0
            for pp in range(PAIRS):
                for t in range(NT):
                    for b in range(2 * pp, 2 * pp + 2):
                        p, half = divmod(b, 2)
                        q = half * 64
                        if t < NTA:
                            src = adjA[BPC * t + b : BPC * t + b + 1, :]
                        else:
                            tb = t - NTA
                            src = adjB[BPC * tb + b : BPC * tb + b + 1, :]
                        scat_last = qeng[i % 3].dma_start(
                            out=wblk[p][q : q + 64, t * 128 + q : t * 128 + q + 64],
                            in_=src,
                        )
                        i += 1

            # pair-1 x chunks after the scatters
            for k in range(len(XC1) - 1):
                load_x_chunk(1, XC1[k], XC1[k + 1], scat_last)

            # filler warmups: keep the PE (and the HAM activity window) busy
            # while the adj copies/scatters drain, so the conv starts warm
            for i in range(NWARM_PRE, NWARM_PRE + 6):
                warmup(i)

            # ---- stage B: per-pair conv ----
            for p in range(PAIRS):
                xp3 = xps[p][:].rearrange("p (h w) -> p h w", w=WP)
                for g in range(NGRP):
                    pos = [
                        ps.tile([128, 512], f32, tag="ps", bufs=8, name=f"po{p}_{g}_{j}")
                        for j in range(4)
                    ]
                    for t in range(NT):
                        kh, kw = divmod(t, 3)
                        for j in range(4):
                            h0 = (g * 4 + j) * 4
                            nc.tensor.matmul(
                                pos[j][:],
                                wblk[p][:, t * 128 : (t + 1) * 128],
                                xp3[:, h0 + kh : h0 + kh + 4, kw : kw + W],
                                start=(t == 0),
                                stop=(t == NT - 1),
                            )
                    # bias-add alternates vector/scalar so the drain of the
                    # final groups is not serialized on one engine
                    os = opool.tile([128, 2048], f32, name=f"os{p}_{g}", tag="os")
                    for j in range(4):
                        dst = os[:, j * 512 : (j + 1) * 512]
                        if j % 2 == 0:
                            nc.vector.tensor_scalar_add(
                                dst, pos[j][:], cst_sb[:, 124:125]
                            )
                        else:
                            nc.scalar.activation(
                                out=dst,
                                in_=pos[j][:],
                                func=AF.Identity,
                                bias=cst_sb[:, 124:125],
                            )
                    last = p == PAIRS - 1 and g == NGRP - 1
                    if last:
                        # per-chunk stores for the final group: each fires
                        # right after its bias-add so the store transfers
                        # overlap the trailing adds instead of serializing
                        for j in range(4):
                            h0 = (g * 4 + j) * 4
                            nc.sync.dma_start(
                                out=out_d.ap()[
                                    2 * p : 2 * p + 2, :, h0 : h0 + 4, :
                                ],
                                in_=os[:, j * 512 : (j + 1) * 512],
                            )
                    else:
                        nc.sync.dma_start(
                            out=out_d.ap()[
                                2 * p : 2 * p + 2, :, g * 16 : (g + 1) * 16, :
                            ],
                            in_=os[:],
                        )

    nc.compile()
    return nc


def _get_nc():
    if "nc" not in _CACHE:
        _CACHE["nc"] = _build()
    return _CACHE["nc"]


def _pack_tables(conv_w, conv_b, mlp_w1, mlp_b1, mlp_w2, mlp_b2):
    """Host-side packing of the learned params (shared across cores)."""
    # T[kk, t, ci, co]: rows 0..15 = mlp_w2 permuted, row 16 = mlp_b2 + conv_w
    w2p = mlp_w2.reshape(MH, COUT, CIN, NT).transpose(0, 3, 2, 1)  # [16,t,ci,co]
    b2p = mlp_b2.reshape(COUT, CIN, NT).transpose(2, 1, 0)         # [t,ci,co]
    cwp = conv_w.reshape(COUT, CIN, NT).transpose(2, 1, 0)
    T = np.concatenate([w2p, (b2p + cwp)[None]], axis=0)           # [17,t,ci,co]
    w2a = np.ascontiguousarray(
        T[:, :NTA].transpose(1, 0, 2, 3).reshape(KA, CC)
    ).astype(BF16)
    w2b = np.ascontiguousarray(
        T[:, NTA:].transpose(1, 0, 2, 3).reshape(KB, CC)
    ).astype(BF16)

    # packed consts [128, 162]: [0:9, 0:4]=ct' (per-core, filled later),
    # [0:9, 4:123]=w1' replicated 7x (cols 17t+kk = w1'[:, kk]),
    # [0:119, 123]=b1' tiled 7x, [:, 124]=cb2,
    # [0:119, 126:154]=block-diag mask for htsA, [0:34, 154:162]=mask for htsB
    cst = np.zeros((128, 162), dtype=np.float32)
    w1p = np.zeros((CL1, K2), dtype=np.float32)
    w1p[:CL, :MH] = mlp_w1
    w1p[CL, MH] = 1.0
    b1p = np.concatenate([mlp_b1, np.zeros(1, np.float32)])
    for t in range(NTA):
        cst[:CL1, 4 + K2 * t : 4 + K2 * (t + 1)] = w1p
        cst[K2 * t : K2 * (t + 1), 123] = b1p
        cst[K2 * t : K2 * (t + 1), 126 + BPC * t : 126 + BPC * (t + 1)] = 1.0
    for t in range(NTB):
        cst[K2 * t : K2 * (t + 1), 154 + BPC * t : 154 + BPC * (t + 1)] = 1.0
    cst[:, 124] = np.tile(conv_b.reshape(COUT), 2)
    return w2a, w2b, cst


def _prep(x, c, conv_w, conv_b, mlp_w1, mlp_b1, mlp_w2, mlp_b2):
    x = np.ascontiguousarray(x, dtype=np.float32)
    c = np.ascontiguousarray(c, dtype=np.float32)
    conv_w = np.asarray(conv_w, dtype=np.float32)
    conv_b = np.asarray(conv_b, dtype=np.float32)
    mlp_w1 = np.asarray(mlp_w1, dtype=np.float32)
    mlp_b1 = np.asarray(mlp_b1, dtype=np.float32)
    mlp_w2 = np.asarray(mlp_w2, dtype=np.float32)
    mlp_b2 = np.asarray(mlp_b2, dtype=np.float32)

    w2a, w2b, cst0 = _pack_tables(conv_w, conv_b, mlp_w1, mlp_b1, mlp_w2, mlp_b2)

    # padded x, flattened spatial, bf16
    xsp = np.zeros((B, CIN, HP, WP), dtype=BF16)
    xsp[:, :, 1 : HP - 1, 1 : WP - 1] = x.astype(BF16)
    xsp = xsp.reshape(B, CIN, HP * WP)

    in_maps = []
    for i in range(NCORES):
        sl = slice(i * BPC, (i + 1) * BPC)
        cst = cst0.copy()
        cst[:CL, 0:4] = c[sl].T
        cst[CL, 0:4] = 1.0
        in_maps.append(
            {
                "xsp": np.ascontiguousarray(xsp[sl]),
                "cst": cst,
                "w2a": w2a,
                "w2b": w2b,
            }
        )
    return in_maps


def _run(inputs, trace=False):
    from concourse.bass_utils import run_bass_kernel_spmd

    nc = _get_nc()
    in_maps = _prep(**inputs)
    res = run_bass_kernel_spmd(
        nc, in_maps, core_ids=list(range(NCORES)), trace=trace
    )
    out = np.concatenate([res.results[i]["out"] for i in range(NCORES)], axis=0)
    return out, res


def kernel(**inputs):
    out, _ = _run(inputs, trace=False)
    return out
